# revision 1
# baseline (speedup 1.0000x reference)
"""GNN scatter-mean (SimpleConv mean + self-loop, threshold col 0) on 8 trn2 cores.

Design (per NeuronCore c of 8):
  - owns nodes [12500c, 12500(c+1)); edges bucketed by dst owner (host).
  - only column 0 of x matters: out[i] = (sum_{j->i} s[j] + s[i] > 0), s = x[:,0]
    (degree >= 1 so the mean's sign equals the sum's sign).
  - 8 Q7-core groups by src chunk (12500 each); per group a replicated SBUF
    table of its s-chunk (+ zero slot); ap_gather fetches s[src] per edge in
    dst-sorted order into 16 row-streams per group (host-balanced rows of
    nodes, identical row ranges across groups).
  - custom DVE cumsum over the [128, CROW] canvas; local_scatter extracts
    P at run ends into per-node packed slots; shifted subtract gives per
    (group,row) node partial sums; a [128x16] 0/1 matmul sums the 8 groups;
    add s_own, threshold > 0.
"""
import numpy as np

import concourse.bass as bass
import concourse.bacc as bacc
import concourse.mybir as mybir
import concourse.tile as tile

# ---------------------------------------------------------------- constants
N_NODES = 100000
N_CORES = 8
NN = N_NODES // N_CORES      # 12500 nodes per core
K = 8                        # src-chunk groups (one per Q7 core)
RR = 16                      # rows per group (one per partition in group)
CROW = 3312                  # stream slots per (group,row)
GC = 8                       # gather calls (2 rows per call)
IDXC = 2 * CROW // 16        # per-core idx cols per call (828B slices, 4B-aligned)
NSLOT = 960                  # packed per-node slots per row (8 chunks x 120)
MCH = 120                    # matmul chunk (psum partitions)
ZSLOT = NN                   # table slot holding 0.0
TBL = 12800                  # table free size (2KB-mult padded)
F32 = mybir.dt.float32
I16 = mybir.dt.int16

_CUMSUM_OP = None


def _register_cumsum():
    global _CUMSUM_OP
    if _CUMSUM_OP is not None:
        return _CUMSUM_OP
    import concourse.dve_ops as dve_ops
    from concourse.dve_ops import DveOp, OPS, CUSTOM_DVE_SPECS, _SUB_OPCODE_FOR_NAME
    from concourse.dve_spec import Spec, Src0, scan, AluOp, lower
    from concourse.dve_uop import DveOpSpec

    name = "CUMSUM_ANT_GNN"
    if name in _SUB_OPCODE_FOR_NAME:
        _CUMSUM_OP = next(o for o in OPS if o.name == name)
        return _CUMSUM_OP
    spec = Spec(
        body=scan(AluOp.ADD, Src0),
        reference=lambda in0, in1, s0, s1, imm2: np.cumsum(
            np.asarray(in0, np.float32), axis=-1, dtype=np.float32
        ),
    )
    opcode = 1 + len(OPS)
    shas = {}
    for ver in ("v3", "v4"):
        s = DveOpSpec(name=name, opcode=opcode, uops=lower(spec, ver=ver), rd1_en=False)
        shas[ver] = s.sha(ver)
    op = DveOp(name, spec, subdim=False, uops_sha=shas)
    OPS.append(op)
    CUSTOM_DVE_SPECS[name] = spec
    _SUB_OPCODE_FOR_NAME[name] = opcode
    _CUMSUM_OP = op
    return op


# ---------------------------------------------------------------- device IR
def build_nc(num_devices=N_CORES, repeat=1, debug_taps=False, ablate=()):
    cum_op = _register_cumsum()
    nc = bacc.Bacc("TRN2", target_bir_lowering=False, debug=False,
                   num_devices=num_devices)
    s_chunks = nc.dram_tensor("s_chunks", [K, TBL], F32, kind="ExternalInput")
    gidx = nc.dram_tensor("gidx", [128, GC * IDXC], I16, kind="ExternalInput")
    bidx = nc.dram_tensor("bidx", [128, 2 * CROW], I16, kind="ExternalInput")
    sown = nc.dram_tensor("sown", [MCH, 128], F32, kind="ExternalInput")
    selm = nc.dram_tensor("selm", [128, 16], F32, kind="ExternalInput")
    perm = nc.dram_tensor("perm", [128, 16 * 128], F32, kind="ExternalInput")
    y = nc.dram_tensor("y", [MCH, 128], F32, kind="ExternalOutput")
    taps = {}
    if debug_taps:
        for tn, shp in (("tap_canvas", [128, CROW]), ("tap_pfx", [128, CROW]),
                        ("tap_packed", [128, NSLOT]), ("tap_dif", [128, NSLOT]),
                        ("tap_gout0", [128, CROW])):
            taps[tn] = nc.dram_tensor(tn, shp, F32, kind="ExternalOutput")

    with tile.TileContext(nc) as tc:
        with (
            tc.tile_pool(name="const", bufs=1) as cpool,
            tc.tile_pool(name="gout", bufs=1) as gpool,
            tc.tile_pool(name="work", bufs=1) as wpool,
            tc.tile_pool(name="psum", bufs=1, space="PSUM") as ppool,
        ):
            table = cpool.tile([128, TBL], F32, tag="table")
            # replicate chunk k into partitions 16k..16k+15 with one DMA
            for kk in range(K):
                nc.sync.dma_start(
                    out=table[16 * kk:16 * (kk + 1), :],
                    in_=s_chunks.ap()[kk:kk + 1, :].to_broadcast([16, TBL]))

            gidx_t = cpool.tile([128, 4096], I16, tag="gidx")
            nc.sync.dma_start(out=gidx_t[:, :GC * IDXC], in_=gidx.ap())
            bidx_t = cpool.tile([128, 8192], I16, tag="bidx")
            nc.sync.dma_start(out=bidx_t[:, :2 * CROW], in_=bidx.ap())
            sown_t = cpool.tile([MCH, 512], F32, tag="sown")
            nc.sync.dma_start(out=sown_t[:, :128], in_=sown.ap())
            selm_t = cpool.tile([128, 512], F32, tag="selm")
            nc.sync.dma_start(out=selm_t[:, :16], in_=selm.ap())
            perm_t = cpool.tile([128, 2048], F32, tag="perm")
            nc.sync.dma_start(out=perm_t[:], in_=perm.ap())

            for _rep in range(repeat):
                canvas = wpool.tile([128, 4096], F32, tag="canvas")
                NCH = 8            # psum column chunks of the canvas
                CCH = CROW // NCH  # cols per chunk
                pstiles = []
                for m in range(NCH):
                    cps = ppool.tile([128, CCH], F32, tag=f"cps{m}", name=f"cps{m}")
                    pstiles.append(cps)
                for j in range(GC):
                    gout = gpool.tile([128, 6656], F32, tag="gout")
                    nc.gpsimd.ap_gather(
                        out_ap=gout[:, :2 * CROW],
                        in_ap=table[:],
                        idxs_ap=gidx_t[:, j * IDXC:(j + 1) * IDXC],
                        channels=128,
                        num_elems=TBL,
                        d=1,
                        num_idxs=2 * CROW,
                    )
                    # Pool-engine fence copy: same-engine ordering guarantees the
                    # gather's SBUF writes are drained before this copy reads them;
                    # PE consumes the copy's output, not the gather's.
                    gout2 = gpool.tile([128, 6656], F32, tag="gout2")
                    nc.gpsimd.tensor_copy(out=gout2[:, :2 * CROW],
                                          in_=gout[:, :2 * CROW])
                    for half in (0, 1):
                        r = 2 * j + half
                        for m in range(NCH):
                            nc.tensor.matmul(
                                out=pstiles[m][:],
                                lhsT=perm_t[:, r * 128:(r + 1) * 128],
                                rhs=gout2[:, half * CROW + m * CCH:
                                          half * CROW + (m + 1) * CCH],
                                start=(r == 0), stop=(r == RR - 1),
                            )
                for m in range(NCH):
                    nc.vector.tensor_copy(out=canvas[:, m * CCH:(m + 1) * CCH],
                                          in_=pstiles[m][:])

                pfx = wpool.tile([128, 4096], F32, tag="pfx")
                if "scan" in ablate:
                    nc.vector.memset(pfx[:, :2], 0.0)
                else:
                    nc.vector._custom_dve(cum_op, out=pfx[:, :CROW], in0=canvas[:, :CROW])
                if debug_taps:
                    nc.sync.dma_start(out=taps["tap_canvas"].ap(), in_=canvas[:, :CROW])
                    nc.sync.dma_start(out=taps["tap_pfx"].ap(), in_=pfx[:, :CROW])

                packed = wpool.tile([128, 1024], F32, tag="packed")
                if "ls" in ablate:
                    nc.vector.memset(packed[:, :2], 0.0)
                elif True:
                    nc.gpsimd.local_scatter(
                    out_ap=packed[:, :NSLOT].bitcast(I16),
                    data_ap=pfx[:, :CROW].bitcast(I16),
                    idxs_ap=bidx_t[:, :2 * CROW],
                    channels=128,
                    num_elems=2 * NSLOT,
                    num_idxs=2 * CROW,
                )

                if debug_taps:
                    nc.sync.dma_start(out=taps["tap_packed"].ap(), in_=packed[:, :NSLOT])
                dif = wpool.tile([128, 1024], F32, tag="dif")
                nc.vector.tensor_copy(out=dif[:, 0:1], in_=packed[:, 0:1])
                nc.vector.tensor_tensor(
                    out=dif[:, 1:NSLOT], in0=packed[:, 1:NSLOT],
                    in1=packed[:, 0:NSLOT - 1], op=mybir.AluOpType.subtract,
                )

                if debug_taps:
                    nc.sync.dma_start(out=taps["tap_dif"].ap(), in_=dif[:, :NSLOT])
                accs = wpool.tile([MCH, 512], F32, tag="accs")
                for m in range(NSLOT // MCH):
                    ps = ppool.tile([MCH, 16], F32, tag=f"cps{m}", name=f"ps{m}")
                    nc.tensor.matmul(
                        out=ps[:], lhsT=dif[:, m * MCH:(m + 1) * MCH],
                        rhs=selm_t[:, :16], start=True, stop=True,
                    )
                    nc.vector.tensor_add(
                        out=accs[:, m * 16:(m + 1) * 16], in0=ps[:],
                        in1=sown_t[:, m * 16:(m + 1) * 16],
                    )

                yt = wpool.tile([MCH, 512], F32, tag="yt")
                nc.vector.tensor_scalar(
                    out=yt[:, :128], in0=accs[:, :128], scalar1=0.0, scalar2=None,
                    op0=mybir.AluOpType.is_gt,
                )
                nc.sync.dma_start(out=y.ap(), in_=yt[:, :128])

    nc.compile()
    return nc


# ---------------------------------------------------------------- host prep
def _permmat():
    pm = np.zeros((128, 16, 128), np.float32)
    for r in range(RR):
        for k in range(K):
            pm[16 * k + r, r, k + 8 * r] = 1.0
    return pm.reshape(128, 16 * 128)


def _selmat():
    m = np.zeros((128, 16), np.float32)
    m[np.arange(128), np.arange(128) // 8] = 1.0
    return m


def prep_core(src_c, dst_c, s, core):
    """src_c: global src ids, dst_c: local dst ids [0,NN); s: full [100000] f32."""
    k = src_c // NN
    srcl = (src_c - k * NN).astype(np.int64)

    cnt = np.bincount(dst_c * K + k, minlength=NN * K).reshape(NN, K)
    absent = cnt == 0
    load_d = cnt.sum(1) + absent.sum(1)
    cum = np.cumsum(load_d)
    total = int(cum[-1])
    targets = total * np.arange(1, RR) / RR
    Rb = np.concatenate([[0], np.searchsorted(cum, targets, side="left") + 1,
                         [NN]]).astype(np.int64)
    rowcounts = np.diff(Rb)
    assert rowcounts.min() > 0 and rowcounts.max() <= NSLOT, rowcounts
    row_of_node = np.repeat(np.arange(RR), rowcounts)

    ad, ak = np.nonzero(absent)
    src_all = np.concatenate([srcl, np.full(len(ad), ZSLOT, np.int64)])
    dst_all = np.concatenate([dst_c, ad])
    k_all = np.concatenate([k, ak])
    row_all = row_of_node[dst_all]
    kr = k_all * RR + row_all
    key = kr * NN + dst_all
    order = np.argsort(key, kind="stable")
    src_s = src_all[order]
    dst_s = dst_all[order]
    key_s = key[order]
    kr_s = kr[order]

    kr_counts = np.bincount(kr_s, minlength=K * RR)
    assert kr_counts.max() <= CROW, kr_counts.max()
    kr_starts = np.concatenate([[0], np.cumsum(kr_counts)])[:-1]
    pos = np.arange(len(kr_s)) - kr_starts[kr_s]

    kk = kr_s // RR
    rr_ = kr_s % RR
    gidx = np.full((128, GC * IDXC), ZSLOT, np.int16)
    ii = (rr_ % 2) * CROW + pos
    gidx[16 * kk + (ii % 16), IDXC * (rr_ // 2) + ii // 16] = src_s.astype(np.int16)

    is_end = np.ones(len(key_s), bool)
    is_end[:-1] = key_s[1:] != key_s[:-1]
    e = np.nonzero(is_end)[0]
    e_k, e_r = kr_s[e] // RR, kr_s[e] % RR
    slot = (dst_s[e] - Rb[e_r]).astype(np.int64)
    bidx = np.full((128, 2 * CROW), -1, np.int16)
    bp = e_k + 8 * e_r
    bidx[bp, 2 * pos[e]] = (2 * slot).astype(np.int16)
    bidx[bp, 2 * pos[e] + 1] = (2 * slot + 1).astype(np.int16)

    s_own = s[core * NN:(core + 1) * NN]
    sown = np.zeros((MCH, 128), np.float32)
    for r in range(RR):
        n0, n1 = int(Rb[r]), int(Rb[r + 1])
        for m in range(NSLOT // MCH):
            lo = n0 + m * MCH
            c_ = min(MCH, n1 - lo)
            if c_ > 0:
                sown[:c_, m * 16 + r] = s_own[lo:lo + c_]
    return gidx, bidx, sown, Rb


def decode_core(yc, Rb):
    out = np.zeros(NN, np.int64)
    for r in range(RR):
        n0, n1 = int(Rb[r]), int(Rb[r + 1])
        for m in range(NSLOT // MCH):
            lo = n0 + m * MCH
            c_ = min(MCH, n1 - lo)
            if c_ > 0:
                out[lo:lo + c_] = (yc[:c_, m * 16 + r] > 0.5).astype(np.int64)
    return out


def prep_all(x, edge_index):
    s = np.asarray(x[:, 0], np.float32)
    src = np.asarray(edge_index[0], np.int64)
    dst = np.asarray(edge_index[1], np.int64)
    owner = dst // NN
    sel_order = np.argsort(owner, kind="stable")
    bounds = np.searchsorted(owner[sel_order], np.arange(N_CORES + 1))
    selm = _selmat()
    permm = _permmat()
    s_chunks = np.zeros((K, TBL), np.float32)
    s_chunks[:, :NN] = s.reshape(K, NN)
    in_maps, infos = [], []
    for c in range(N_CORES):
        idx = sel_order[bounds[c]:bounds[c + 1]]
        gidx, bidx, sown, Rb = prep_core(src[idx], dst[idx] - c * NN, s, c)
        in_maps.append({
            "s_chunks": s_chunks, "gidx": gidx, "bidx": bidx,
            "sown": sown, "selm": selm, "perm": permm,
        })
        infos.append(Rb)
    return in_maps, infos


def decode_all(results, infos):
    return np.concatenate(
        [decode_core(results[c]["y"], infos[c]) for c in range(N_CORES)])


# ------------------------------------------------------------- numpy model
def numpy_model_core(in_map):
    """Bit-for-bit-ish model of the device pipeline for one core (f32 order
    matches: sequential scan, diffs, 8-way group sum)."""
    s_chunks = in_map["s_chunks"]
    table = np.zeros((128, TBL), np.float32)
    for p in range(128):
        table[p, :] = s_chunks[p // 16]
    gidx = in_map["gidx"]
    canvas = np.zeros((128, CROW), np.float32)
    for r in range(RR):
        sl = gidx[:, (r // 2) * IDXC:(r // 2 + 1) * IDXC]
        for k in range(K):
            idxs = sl[16 * k:16 * (k + 1), :].T.reshape(-1)  # (s p), 2*CROW long
            idxs = idxs[(r % 2) * CROW:(r % 2 + 1) * CROW]
            canvas[k + 8 * r, :] = table[16 * k + r, idxs]
    pfx = np.cumsum(canvas, axis=1, dtype=np.float32)
    packed = np.zeros((128, NSLOT), np.float32)
    pk16 = packed.view(np.int16).reshape(128, 2 * NSLOT)
    pf16 = pfx.view(np.int16).reshape(128, 2 * CROW)
    bidx = in_map["bidx"]
    for p in range(128):
        v = bidx[p] >= 0
        pk16[p, bidx[p][v].astype(np.int64)] = pf16[p, np.nonzero(v)[0]]
    dif = np.zeros((128, NSLOT), np.float32)
    dif[:, 0] = packed[:, 0]
    dif[:, 1:] = packed[:, 1:] - packed[:, :-1]
    accs = np.zeros((MCH, 128), np.float32)
    selm = in_map["selm"]
    for m in range(NSLOT // MCH):
        ps = dif[:, m * MCH:(m + 1) * MCH].T @ selm
        accs[:, m * 16:(m + 1) * 16] = ps + in_map["sown"][:, m * 16:(m + 1) * 16]
    return (accs > 0).astype(np.float32)


# ---------------------------------------------------------------- entrypoint
_NC_CACHE = {}


def kernel(x, edge_index):
    """Full inputs in, full output out; shards across 8 NeuronCores inside."""
    from concourse.bass_utils import run_bass_kernel_spmd
    x = np.asarray(x)
    edge_index = np.asarray(edge_index)
    in_maps, infos = prep_all(x, edge_index)
    if "nc" not in _NC_CACHE:
        _NC_CACHE["nc"] = build_nc(num_devices=N_CORES)
    res = run_bass_kernel_spmd(_NC_CACHE["nc"], in_maps,
                               core_ids=list(range(N_CORES)))
    out = decode_all(res.results, infos)
    return out.astype(np.int64)

